# revision 2
# baseline (speedup 1.0000x reference)
"""Multi-head causal attention (B=2, T=2048, H=1024, 16 heads) on 8 Trainium2
NeuronCores — collective-free sequence-parallel sharding.

Each core owns 512 query rows of one batch: core = 4*b + j handles 128-row
query blocks {j, j+4, j+8, j+12} (strided so causal work is balanced).  Every
core redundantly computes the full K/V projection (+rope/mask folding) for its
batch, projects Q for its own rows, runs attention and the FULL out-projection
for its rows, and writes a disjoint [512, 1024] slice of the output.  No
inter-core communication at all; the host merely places each slice.  The
per-core causal structure (which key chunk is each block's diagonal) is pure
input data, so one SPMD program serves all cores.

Layout: scores computed transposed (keys on partitions, queries free) so
softmax'd tiles feed PV directly; V carries a per-head ones-column (= the
padding-mask value) so PV also yields softmax denominators.  Q/K pair-packed
two heads per 128 partitions; score stationaries are zero-padded per head.

Self-contained: shapes/sharding hardcoded; only needs the concourse runtime.
"""
import sys

for _p in ("/opt/trn_rl_repo", "/root/.axon_site/_ro/trn_rl_repo"):
    if _p not in sys.path:
        sys.path.append(_p)

from contextlib import ExitStack

import numpy as np
import ml_dtypes

import concourse.bacc as bacc
import concourse.tile as tile
from concourse import mybir
from concourse.bass_utils import run_bass_kernel_spmd

BF16 = ml_dtypes.bfloat16
F32 = mybir.dt.float32
BF = mybir.dt.bfloat16

N_CORES = 8
B, T, H = 2, 2048, 1024
NH, HD = 16, 64
NP = NH // 2          # 8 head pairs
NKC = T // 128        # 16 key chunks
NQB = 4               # query blocks per core (128 rows each)
ROPE_BASE = 10000.0

_PROGRAMS = {}


def _emit_body(ctx, tc, io, pools, phases=(1, 2, 3)):
    nc = tc.nc
    mult = mybir.AluOpType.mult
    add = mybir.AluOpType.add
    AF = mybir.ActivationFunctionType

    c = pools["consts"]
    ps_sp = pools["ps_sp"]
    ps_mm = pools["ps_mm"]
    ps_oa = pools["ps_oa"]
    p_kraw = pools["kraw"]
    p_sh = pools["sh"]
    p_t2 = pools["t2"]
    p_kz = pools["kz"]
    p_qraw = pools["qraw"]
    p_E = pools["E"]
    p_osb = pools["osb"]
    p_ot = pools["ot"]
    p_ysb = pools["ysb"]
    y = io["y"]
    onp = io["onp"]
    v_sb = io["v_sb"]
    qf = io["qf"]

    # ---- phase 1a: q projection + bias + rope (per pair, own 512 rows) ----
    for p in range(NP if 1 in phases else 0):
        ps = ps_mm.tile([128, 512], F32, tag="mm", name="psq")
        for k in range(8):
            nc.tensor.matmul(ps[:], c["wqk"][:, k, 128 * p:128 * p + 128],
                             c["xq"][:, k, :], start=(k == 0), stop=(k == 7))
        qraw = p_qraw.tile([128, 512], BF, tag="qraw")
        nc.vector.tensor_scalar(qraw[:], ps[:], c["bqk"][:, p:p + 1],
                                None, add)
        shq = p_sh.tile([128, 512], BF, tag="sh", name="shq")
        for dst, src in ((0, 32), (32, 0), (64, 96), (96, 64)):
            nc.sync.dma_start(shq[dst:dst + 32, :], qraw[src:src + 32, :])
        t2q = p_t2.tile([128, 512], BF, tag="t2", name="t2q")
        nc.vector.tensor_tensor(t2q[:], shq[:], c["sinQ"][:], mult)
        nc.vector.tensor_tensor(qf[:, p, :], qraw[:], c["cosQ"][:], mult)
        nc.vector.tensor_tensor(qf[:, p, :], qf[:, p, :], t2q[:], add)

    # ---- phase 1b: v projection (all heads) + bias, mask folded in -------
    for kc in range(NKC if 1 in phases else 0):
        psv = ps_mm.tile([128, 2, 512], F32, tag="mm", name="psv")
        for h2 in range(2):
            nc.tensor.matmul(psv[:, h2, :], c["ones1"][0:1, :],
                             c["bv"][0:1, 512 * h2:512 * h2 + 512],
                             start=True, stop=False)
            for k in range(8):
                nc.tensor.matmul(
                    psv[:, h2, :], c["xT"][:, k, 128 * kc:128 * kc + 128],
                    c["wv"][:, k, 512 * h2:512 * h2 + 512],
                    start=False, stop=(k == 7))
        vv = v_sb[:, kc, :].rearrange("p (h d) -> p h d", h=NH)
        nc.vector.tensor_scalar(
            vv[:, :, 0:64],
            psv[:].rearrange("p h2 (h8 d) -> p (h2 h8) d", d=64),
            c["km"][:, kc:kc + 1], None, mult)
        nc.vector.tensor_scalar(vv[:, :, 64:65], c["ones16"][:],
                                c["km"][:, kc:kc + 1], None, mult)

    # ---- phase 1c + 2: per pair: k proj + rope, then attention ------------
    for p in range(NP if 2 in phases else 0):
        kraw = p_kraw.tile([128, T], BF, tag="kraw")
        for half in range(2):
            ps = ps_mm.tile([128, 2, 512], F32, tag="mm", name="psk")
            for sub in range(2):
                for k in range(8):
                    nc.tensor.matmul(
                        ps[:, sub, :],
                        c["wqk"][:, k, 1024 + 128 * p:1024 + 128 * p + 128],
                        c["xT"][:, k, 1024 * half + 512 * sub:
                                1024 * half + 512 * sub + 512],
                        start=(k == 0), stop=(k == 7))
            nc.vector.tensor_scalar(
                kraw[:, 1024 * half:1024 * half + 1024],
                ps[:].rearrange("p a b -> p (a b)"),
                c["bqk"][:, 8 + p:9 + p], None, add)
        sh = p_sh.tile([128, T], BF, tag="sh")
        for dst, src in ((0, 32), (32, 0), (64, 96), (96, 64)):
            nc.sync.dma_start(sh[dst:dst + 32, :], kraw[src:src + 32, :])
        t2 = p_t2.tile([128, T], BF, tag="t2")
        nc.vector.tensor_tensor(t2[:], sh[:], c["sinK"][:], mult)
        kz = []
        for e in range(2):
            kze = p_kz.tile([128, T], BF, tag=f"kz{e}", name=f"kz{e}")
            lo, hi = 64 * e, 64 * e + 64
            nc.vector.memset(kze[64 - 64 * e:128 - 64 * e, :], 0.0)
            nc.vector.tensor_tensor(kze[lo:hi, :], kraw[lo:hi, :],
                                    c["cosK"][lo:hi, :], mult)
            nc.vector.tensor_tensor(kze[lo:hi, :], kze[lo:hi, :],
                                    t2[lo:hi, :], add)
            kz.append(kze)

        # attention for each of the 4 query blocks of this pair
        for i in range(NQB):
            oa = [ps_oa.tile([65, 128], F32, tag=f"oa{e}", name=f"oa{e}")
                  for e in range(2)]
            nch = 4 * i + 4
            for g in range(i + 1):
                sp = ps_sp.tile([128, 4, 2, 128], F32, tag="sp", name="sp")
                for t in range(4):
                    for e in range(2):
                        nc.tensor.matmul(
                            sp[:, t, e, :],
                            kz[e][:, 128 * (4 * g + t):128 * (4 * g + t) + 128],
                            qf[:, p, 128 * i:128 * i + 128],
                            start=True, stop=True)
                E = p_E.tile([128, 4, 2, 128], BF, tag="E")
                nc.scalar.activation(E[:], sp[:], AF.Exp, scale=0.125)
                if g == i:  # diagonal group: per-core causal strip
                    nc.gpsimd.tensor_tensor(
                        E[:], E[:],
                        c["caus"][:, :].rearrange("p (t e q) -> p t e q",
                                                  t=4, e=2), mult)
                for t in range(4):
                    for e in range(2):
                        ck = 4 * g + t
                        nc.tensor.matmul(
                            oa[e][:],
                            v_sb[:, ck, 65 * (2 * p + e):65 * (2 * p + e) + 65],
                            E[:, t, e, :],
                            start=(ck == 0), stop=(ck == nch - 1))
            osb = p_osb.tile([65, 2, 128], F32, tag="osb")
            for e in range(2):
                nc.vector.tensor_copy(osb[:, e, :], oa[e][:])
            nc.vector.reciprocal(osb[64:65, :, :], osb[64:65, :, :])
            bc = ps_mm.tile([64, 2, 128], F32, tag="mm", name="bc")
            for e in range(2):
                nc.tensor.matmul(bc[:, e, :], c["ones64"][64:65, 0:64],
                                 osb[64:65, e, :], start=True, stop=True)
            nc.vector.tensor_tensor(onp[0:64, p, 128 * i:128 * i + 128],
                                    osb[0:64, 0, :], bc[:, 0, :], mult)
            ot = p_ot.tile([64, 128], BF, tag="ot")
            nc.vector.tensor_tensor(ot[:], osb[0:64, 1, :], bc[:, 1, :],
                                    mult)
            nc.sync.dma_start(onp[64:128, p, 128 * i:128 * i + 128], ot[:])

    # ---- phase 3: out-projection (full H for own 512 rows) ---------------
    for s in range(NQB if 3 in phases else 0):
        for ns in range(2):
            py = ps_mm.tile([128, 512], F32, tag="mm", name="py")
            for p in range(NP):
                nc.tensor.matmul(
                    py[:], onp[:, p, 128 * s:128 * s + 128],
                    c["wr"][:, p, 512 * ns:512 * ns + 512],
                    start=(p == 0), stop=(p == NP - 1))
            ysb = p_ysb.tile([128, 512], F32, tag="ysb")
            nc.vector.tensor_copy(ysb[:], py[:])
            nc.sync.dma_start(
                y[128 * s:128 * s + 128, 512 * ns:512 * ns + 512], ysb[:])


def build_program(nreps=1, use_collective=True, phases=(1, 2, 3)):
    key = (nreps, tuple(phases))
    if key in _PROGRAMS:
        return _PROGRAMS[key]

    nc = bacc.Bacc("TRN2", target_bir_lowering=False, debug=False,
                   num_devices=N_CORES)
    xT = nc.dram_tensor("xT", [H, T], BF, kind="ExternalInput")
    xq = nc.dram_tensor("xq", [H, 512], BF, kind="ExternalInput")
    wqk = nc.dram_tensor("wqk", [H, 2048], BF, kind="ExternalInput")
    wv = nc.dram_tensor("wv", [H, 1024], BF, kind="ExternalInput")
    wr = nc.dram_tensor("wr", [NP, 128, H], BF, kind="ExternalInput")
    bqkT = nc.dram_tensor("bqkT", [128, 16], F32, kind="ExternalInput")
    bv = nc.dram_tensor("bv", [1, 1024], BF, kind="ExternalInput")
    cosK = nc.dram_tensor("cosK", [128, T], BF, kind="ExternalInput")
    sinK = nc.dram_tensor("sinK", [128, T], BF, kind="ExternalInput")
    cosQ = nc.dram_tensor("cosQ", [128, 512], BF, kind="ExternalInput")
    sinQ = nc.dram_tensor("sinQ", [128, 512], BF, kind="ExternalInput")
    caus = nc.dram_tensor("caus", [128, 1024], BF, kind="ExternalInput")
    kmT = nc.dram_tensor("kmT", [128, NKC], F32, kind="ExternalInput")
    yout = nc.dram_tensor("y", [512, H], F32, kind="ExternalOutput")

    with tile.TileContext(nc) as tc, ExitStack() as ctx:
        const = ctx.enter_context(tc.tile_pool(name="const", bufs=1))
        ps_sp = ctx.enter_context(tc.tile_pool(name="ps_sp", bufs=2,
                                               space="PSUM"))
        ps_mm = ctx.enter_context(tc.tile_pool(name="ps_mm", bufs=1,
                                               space="PSUM"))
        ps_oa = ctx.enter_context(tc.tile_pool(name="ps_oa", bufs=1,
                                               space="PSUM"))
        p_kraw = ctx.enter_context(tc.tile_pool(name="kraw", bufs=1))
        p_sh = ctx.enter_context(tc.tile_pool(name="sh", bufs=1))
        p_t2 = ctx.enter_context(tc.tile_pool(name="t2", bufs=1))
        p_kz = ctx.enter_context(tc.tile_pool(name="kz", bufs=2))
        p_qraw = ctx.enter_context(tc.tile_pool(name="qraw", bufs=2))
        p_E = ctx.enter_context(tc.tile_pool(name="E", bufs=2))
        p_osb = ctx.enter_context(tc.tile_pool(name="osb", bufs=2))
        p_ot = ctx.enter_context(tc.tile_pool(name="ot", bufs=2))
        p_ysb = ctx.enter_context(tc.tile_pool(name="ysb", bufs=2))

        cst = {}
        cst["xT"] = const.tile([128, 8, T], BF, name="xT_sb")
        nc.sync.dma_start(cst["xT"][:],
                          xT.ap().rearrange("(k p) t -> p k t", p=128))
        cst["xq"] = const.tile([128, 8, 512], BF, name="xq_sb")
        nc.sync.dma_start(cst["xq"][:],
                          xq.ap().rearrange("(k p) t -> p k t", p=128))
        cst["wqk"] = const.tile([128, 8, 2048], BF, name="wqk_sb")
        nc.sync.dma_start(cst["wqk"][:],
                          wqk.ap().rearrange("(k p) m -> p k m", p=128))
        cst["wv"] = const.tile([128, 8, 1024], BF, name="wv_sb")
        nc.sync.dma_start(cst["wv"][:],
                          wv.ap().rearrange("(k p) m -> p k m", p=128))
        cst["wr"] = const.tile([128, NP, H], BF, name="wr_sb")
        nc.sync.dma_start(cst["wr"][:], wr.ap().rearrange("h p m -> p h m"))
        cst["bqk"] = const.tile([128, 16], F32, name="bqk_sb")
        nc.sync.dma_start(cst["bqk"][:], bqkT.ap())
        cst["bv"] = const.tile([1, 1024], BF, name="bv_sb")
        nc.sync.dma_start(cst["bv"][:], bv.ap())
        for nm, dt_ in (("cosK", cosK), ("sinK", sinK), ("cosQ", cosQ),
                        ("sinQ", sinQ), ("caus", caus)):
            cst[nm] = const.tile(list(dt_.shape), BF, name=f"{nm}_sb")
            nc.sync.dma_start(cst[nm][:], dt_.ap())
        cst["km"] = const.tile([128, NKC], F32, name="km_sb")
        nc.sync.dma_start(cst["km"][:], kmT.ap())
        cst["ones1"] = const.tile([1, 128], BF, name="ones1")
        nc.vector.memset(cst["ones1"][:], 1.0)
        cst["ones16"] = const.tile([128, NH, 1], BF, name="ones16")
        nc.vector.memset(cst["ones16"][:], 1.0)
        cst["ones64"] = const.tile([128, 64], F32, name="ones64")
        nc.vector.memset(cst["ones64"][:], 1.0)

        v_sb = const.tile([128, NKC, NH * 65], BF, name="v_sb")
        qf = const.tile([128, NP, 512], BF, name="qf_sb")
        onp = const.tile([128, NP, 512], BF, name="onp_sb")

        pools = dict(
            consts=cst, ps_sp=ps_sp, ps_mm=ps_mm, ps_oa=ps_oa,
            kraw=p_kraw, sh=p_sh, t2=p_t2, kz=p_kz, qraw=p_qraw, E=p_E,
            osb=p_osb, ot=p_ot, ysb=p_ysb,
        )
        io = dict(y=yout.ap(), onp=onp, v_sb=v_sb, qf=qf)

        for _ in range(nreps):
            _emit_body(ctx, tc, io, pools, phases=phases)

    nc.compile()
    _PROGRAMS[key] = nc
    return nc


def make_inputs(hidden_state, attention_mask, w_qkv, b_qkv, w_out):
    """Host-side shard prep. Returns one input dict per core."""
    hidden_state = np.asarray(hidden_state)
    attention_mask = np.asarray(attention_mask)
    w_qkv = np.asarray(w_qkv)
    b_qkv = np.asarray(b_qkv)
    w_out = np.asarray(w_out)

    # rope tables (fp32 as in the reference, then bf16 for the device)
    inv_freq = 1.0 / (ROPE_BASE ** (np.arange(0, HD, 2, dtype=np.float32)
                                    / HD))
    t = np.arange(T, dtype=np.float32)
    freqs = np.outer(t, inv_freq)                      # [T, 32]
    emb = np.concatenate([freqs, freqs], axis=-1)      # [T, HD]
    cosT = np.cos(emb).T.astype(np.float32)            # [HD, T]
    sinT = np.sin(emb).T.astype(np.float32)
    sin_eff = sinT.copy()
    sin_eff[:32] = -sin_eff[:32]
    cos_pair = np.vstack([cosT, cosT]).astype(BF16)    # [128, T]
    sin_pair = np.vstack([sin_eff, sin_eff]).astype(BF16)

    # weight column orders
    cols_q = np.concatenate([np.arange(h * 192, h * 192 + 64)
                             for h in range(NH)])
    cols_k = cols_q + 64
    cols_v = cols_q + 128
    wqk = np.concatenate([w_qkv[:, cols_q], w_qkv[:, cols_k]],
                         axis=1).astype(BF16)          # [H, 2048]
    bqk = np.concatenate([b_qkv[cols_q], b_qkv[cols_k]]).astype(np.float32)
    bqkT = bqk.reshape(16, 128).T.copy()               # [128, 16]
    wv = w_qkv[:, cols_v].astype(BF16)                 # [H, 1024]
    bvr = b_qkv[cols_v].astype(BF16).reshape(1, 1024)
    wr = w_out.reshape(NP, 128, H).astype(BF16)

    dk = np.arange(128)[:, None]
    dq = np.arange(128)[None, :]

    in_maps = []
    for core in range(N_CORES):
        b, j = core // 4, core % 4
        qrows = np.concatenate([np.arange(128 * j + 512 * i,
                                          128 * j + 512 * i + 128)
                                for i in range(NQB)])
        # causal strips: within key chunk 4g+t of the diagonal group,
        # t < j all-pass, t == j diagonal, t > j fully masked
        caus = np.zeros((128, 4, 2, 128), dtype=BF16)
        for t in range(4):
            if t < j:
                pat = np.ones((128, 128), dtype=BF16)
            elif t == j:
                pat = (dq >= dk).astype(BF16)
            else:
                pat = np.zeros((128, 128), dtype=BF16)
            caus[:, t, 0, :] = pat
            caus[:, t, 1, :] = pat

        kmT = (attention_mask[b].reshape(NKC, 128).T != 0) \
            .astype(np.float32)

        in_maps.append({
            "xT": np.ascontiguousarray(hidden_state[b].T).astype(BF16),
            "xq": np.ascontiguousarray(hidden_state[b][qrows].T)
            .astype(BF16),
            "wqk": wqk,
            "wv": wv,
            "wr": wr,
            "bqkT": bqkT,
            "bv": bvr,
            "cosK": cos_pair,
            "sinK": sin_pair,
            "cosQ": np.ascontiguousarray(cos_pair[:, qrows]),
            "sinQ": np.ascontiguousarray(sin_pair[:, qrows]),
            "caus": caus.reshape(128, 1024),
            "kmT": kmT,
        })
    return in_maps


def kernel(hidden_state, attention_mask, w_qkv, b_qkv, w_out):
    nc = build_program(nreps=1)
    in_maps = make_inputs(hidden_state, attention_mask, w_qkv, b_qkv, w_out)
    res = run_bass_kernel_spmd(nc, in_maps, list(range(N_CORES))).results

    out = np.empty((B, T, H), dtype=np.float32)
    for core in range(N_CORES):
        b, j = core // 4, core % 4
        for i in range(NQB):
            out[b, 128 * j + 512 * i:128 * j + 512 * i + 128, :] = \
                res[core]["y"][128 * i:128 * i + 128, :]
    return out


# revision 3
# speedup vs baseline: 1.1903x; 1.1903x over previous
"""Multi-head causal attention (B=2, T=2048, H=1024, 16 heads) on 8 Trainium2
NeuronCores — collective-free sequence-parallel sharding.

Each core owns 512 query rows of one batch: core = 4*b + j handles 128-row
query blocks {j, j+4, j+8, j+12} (strided so causal work is balanced).  Every
core redundantly computes the full K/V projection (+rope/mask folding) for its
batch, projects Q for its own rows, runs attention and the FULL out-projection
for its rows, and writes a disjoint [512, 1024] slice of the output.  No
inter-core communication at all; the host merely places each slice.  The
per-core causal structure (which key chunk is each block's diagonal) is pure
input data, so one SPMD program serves all cores.

Layout: scores computed transposed (keys on partitions, queries free) so
softmax'd tiles feed PV directly; V carries a per-head ones-column (= the
padding-mask value) so PV also yields softmax denominators.  Q/K pair-packed
two heads per 128 partitions; score stationaries are zero-padded per head.

Self-contained: shapes/sharding hardcoded; only needs the concourse runtime.
"""
import sys

for _p in ("/opt/trn_rl_repo", "/root/.axon_site/_ro/trn_rl_repo"):
    if _p not in sys.path:
        sys.path.append(_p)

from contextlib import ExitStack

import numpy as np
import ml_dtypes

import concourse.bacc as bacc
import concourse.tile as tile
from concourse import mybir
from concourse.bass_utils import run_bass_kernel_spmd

BF16 = ml_dtypes.bfloat16
F32 = mybir.dt.float32
BF = mybir.dt.bfloat16

N_CORES = 8
B, T, H = 2, 2048, 1024
NH, HD = 16, 64
NP = NH // 2          # 8 head pairs
NKC = T // 128        # 16 key chunks
NQB = 4               # query blocks per core (128 rows each)
ROPE_BASE = 10000.0

_PROGRAMS = {}


def _emit_body(ctx, tc, io, pools, phases=(1, 2, 3)):
    nc = tc.nc
    mult = mybir.AluOpType.mult
    add = mybir.AluOpType.add
    AF = mybir.ActivationFunctionType

    c = pools["consts"]
    ps_sp = pools["ps_sp"]
    ps_mm = pools["ps_mm"]
    ps_oa = pools["ps_oa"]
    p_kraw = pools["kraw"]
    p_t2 = pools["t2"]
    p_kz = pools["kz"]
    p_qraw = pools["qraw"]
    p_E = pools["E"]
    p_osb = pools["osb"]
    p_ot = pools["ot"]
    p_ysb = pools["ysb"]
    y = io["y"]
    onp = io["onp"]
    v_sb = io["v_sb"]
    qf = io["qf"]

    # ---- phase 1a: q projection + bias + rope (per pair, own 512 rows) ----
    for p in range(NP if 1 in phases else 0):
        ps = ps_mm.tile([128, 512], F32, tag="mm", name="psq")
        for k in range(8):
            nc.tensor.matmul(ps[:], c["wqk"][:, k, 128 * p:128 * p + 128],
                             c["xq"][:, k, :], start=(k == 0), stop=(k == 7))
        qraw = p_qraw.tile([128, 512], BF, tag="qraw")
        nc.vector.tensor_scalar(qraw[:], ps[:], c["bqk"][:, p:p + 1],
                                None, add)
        shq = ps_mm.tile([128, 512], F32, tag="mm", name="shq")
        nc.tensor.matmul(shq[:], c["prot"][:], qraw[:], start=True,
                         stop=True)
        t2q = p_t2.tile([128, 512], BF, tag="t2", name="t2q")
        nc.vector.tensor_tensor(t2q[:], shq[:], c["sinQ"][:], mult)
        nc.vector.tensor_tensor(qf[:, p, :], qraw[:], c["cosQ"][:], mult)
        nc.vector.tensor_tensor(qf[:, p, :], qf[:, p, :], t2q[:], add)

    # ---- phase 1b: v projection (all heads) + bias, mask folded in -------
    for kc in range(NKC if 1 in phases else 0):
        for q4 in range(4):
            psv = ps_mm.tile([128, 260], F32, tag="mm", name="psv")
            nc.tensor.matmul(psv[:], c["ones1"][0:1, :],
                             c["bv"][0:1, 260 * q4:260 * q4 + 260],
                             start=True, stop=False)
            for k in range(8):
                nc.tensor.matmul(
                    psv[:], c["xT"][:, k, 128 * kc:128 * kc + 128],
                    c["wv"][:, k, 260 * q4:260 * q4 + 260],
                    start=False, stop=(k == 7))
            nc.vector.tensor_scalar(
                v_sb[:, kc, 260 * q4:260 * q4 + 260], psv[:],
                c["km"][:, kc:kc + 1], None, mult)

    # ---- phase 1c + 2: per pair: k proj + rope, then attention ------------
    for p in range(NP if 2 in phases else 0):
        kraw = p_kraw.tile([128, T], BF, tag="kraw")
        for half in range(2):
            ps = ps_mm.tile([128, 2, 512], F32, tag="mm", name="psk")
            for sub in range(2):
                for k in range(8):
                    nc.tensor.matmul(
                        ps[:, sub, :],
                        c["wqk"][:, k, 1024 + 128 * p:1024 + 128 * p + 128],
                        c["xT"][:, k, 1024 * half + 512 * sub:
                                1024 * half + 512 * sub + 512],
                        start=(k == 0), stop=(k == 7))
            nc.vector.tensor_scalar(
                kraw[:, 1024 * half:1024 * half + 1024],
                ps[:].rearrange("p a b -> p (a b)"),
                c["bqk"][:, 8 + p:9 + p], None, add)
        t2 = p_t2.tile([128, T], BF, tag="t2")
        for half in range(2):
            psh = ps_mm.tile([128, 2, 512], F32, tag="mm", name="psh")
            for sub in range(2):
                nc.tensor.matmul(
                    psh[:, sub, :], c["prot"][:],
                    kraw[:, 1024 * half + 512 * sub:
                         1024 * half + 512 * sub + 512],
                    start=True, stop=True)
            nc.vector.tensor_tensor(
                t2[:, 1024 * half:1024 * half + 1024],
                psh[:].rearrange("p a b -> p (a b)"),
                c["sinK"][:, 1024 * half:1024 * half + 1024], mult)
        kz = []
        for e in range(2):
            kze = p_kz.tile([128, T], BF, tag=f"kz{e}", name=f"kz{e}")
            lo, hi = 64 * e, 64 * e + 64
            nc.vector.memset(kze[64 - 64 * e:128 - 64 * e, :], 0.0)
            nc.vector.tensor_tensor(kze[lo:hi, :], kraw[lo:hi, :],
                                    c["cosK"][lo:hi, :], mult)
            nc.vector.tensor_tensor(kze[lo:hi, :], kze[lo:hi, :],
                                    t2[lo:hi, :], add)
            kz.append(kze)

        # attention for each of the 4 query blocks of this pair
        ot = p_ot.tile([64, 512], BF, tag="ot")
        for i in range(NQB):
            oa = [ps_oa.tile([65, 128], F32, tag=f"oa{e}", name=f"oa{e}")
                  for e in range(2)]
            nch = 4 * i + 4
            for g in range(i + 1):
                sp = ps_sp.tile([128, 4, 2, 128], F32, tag="sp", name="sp")
                for t in range(4):
                    for e in range(2):
                        nc.tensor.matmul(
                            sp[:, t, e, :],
                            kz[e][:, 128 * (4 * g + t):128 * (4 * g + t) + 128],
                            qf[:, p, 128 * i:128 * i + 128],
                            start=True, stop=True)
                if g == i:  # diagonal group: per-core causal strip
                    nc.vector.tensor_tensor(
                        sp[:], sp[:],
                        c["caus"][:, :].rearrange("p (t e q) -> p t e q",
                                                  t=4, e=2), add)
                E = p_E.tile([128, 4, 2, 128], BF, tag="E")
                nc.scalar.activation(E[:], sp[:], AF.Exp, scale=0.125)
                for t in range(4):
                    for e in range(2):
                        ck = 4 * g + t
                        nc.tensor.matmul(
                            oa[e][:],
                            v_sb[:, ck, 65 * (2 * p + e):65 * (2 * p + e) + 65],
                            E[:, t, e, :],
                            start=(ck == 0), stop=(ck == nch - 1))
            osb = p_osb.tile([65, 2, 128], F32, tag="osb")
            for e in range(2):
                nc.vector.tensor_copy(osb[:, e, :], oa[e][:])
            nc.vector.reciprocal(osb[64:65, :, :], osb[64:65, :, :])
            bc = ps_mm.tile([64, 2, 128], F32, tag="mm", name="bc")
            for e in range(2):
                nc.tensor.matmul(bc[:, e, :], c["ones64"][64:65, 0:64],
                                 osb[64:65, e, :], start=True, stop=True)
            nc.vector.tensor_tensor(onp[0:64, p, 128 * i:128 * i + 128],
                                    osb[0:64, 0, :], bc[:, 0, :], mult)
            nc.vector.tensor_tensor(ot[:, 128 * i:128 * i + 128],
                                    osb[0:64, 1, :], bc[:, 1, :], mult)
        nc.sync.dma_start(onp[64:128, p, :], ot[:])

    # ---- phase 3: out-projection (full H for own 512 rows) ---------------
    for s in range(NQB if 3 in phases else 0):
        for ns in range(2):
            py = ps_mm.tile([128, 512], F32, tag="mm", name="py")
            for p in range(NP):
                nc.tensor.matmul(
                    py[:], onp[:, p, 128 * s:128 * s + 128],
                    c["wr"][:, p, 512 * ns:512 * ns + 512],
                    start=(p == 0), stop=(p == NP - 1))
            ysb = p_ysb.tile([128, 512], F32, tag="ysb")
            nc.vector.tensor_copy(ysb[:], py[:])
            nc.sync.dma_start(
                y[128 * s:128 * s + 128, 512 * ns:512 * ns + 512], ysb[:])


def build_program(nreps=1, use_collective=True, phases=(1, 2, 3)):
    key = (nreps, tuple(phases))
    if key in _PROGRAMS:
        return _PROGRAMS[key]

    nc = bacc.Bacc("TRN2", target_bir_lowering=False, debug=False,
                   num_devices=N_CORES)
    xT = nc.dram_tensor("xT", [H, T], BF, kind="ExternalInput")
    xq = nc.dram_tensor("xq", [H, 512], BF, kind="ExternalInput")
    wqk = nc.dram_tensor("wqk", [H, 2048], BF, kind="ExternalInput")
    wv = nc.dram_tensor("wv", [H, 1040], BF, kind="ExternalInput")
    wr = nc.dram_tensor("wr", [NP, 128, H], BF, kind="ExternalInput")
    bqkT = nc.dram_tensor("bqkT", [128, 16], F32, kind="ExternalInput")
    bv = nc.dram_tensor("bv", [1, 1040], BF, kind="ExternalInput")
    prot = nc.dram_tensor("prot", [128, 128], BF, kind="ExternalInput")
    cosK = nc.dram_tensor("cosK", [128, T], BF, kind="ExternalInput")
    sinK = nc.dram_tensor("sinK", [128, T], BF, kind="ExternalInput")
    cosQ = nc.dram_tensor("cosQ", [128, 512], BF, kind="ExternalInput")
    sinQ = nc.dram_tensor("sinQ", [128, 512], BF, kind="ExternalInput")
    caus = nc.dram_tensor("caus", [128, 1024], BF, kind="ExternalInput")
    kmT = nc.dram_tensor("kmT", [128, NKC], F32, kind="ExternalInput")
    yout = nc.dram_tensor("y", [512, H], F32, kind="ExternalOutput")

    with tile.TileContext(nc) as tc, ExitStack() as ctx:
        const = ctx.enter_context(tc.tile_pool(name="const", bufs=1))
        ps_sp = ctx.enter_context(tc.tile_pool(name="ps_sp", bufs=2,
                                               space="PSUM"))
        ps_mm = ctx.enter_context(tc.tile_pool(name="ps_mm", bufs=1,
                                               space="PSUM"))
        ps_oa = ctx.enter_context(tc.tile_pool(name="ps_oa", bufs=1,
                                               space="PSUM"))
        p_kraw = ctx.enter_context(tc.tile_pool(name="kraw", bufs=1))
        p_t2 = ctx.enter_context(tc.tile_pool(name="t2", bufs=1))
        p_kz = ctx.enter_context(tc.tile_pool(name="kz", bufs=2))
        p_qraw = ctx.enter_context(tc.tile_pool(name="qraw", bufs=2))
        p_E = ctx.enter_context(tc.tile_pool(name="E", bufs=2))
        p_osb = ctx.enter_context(tc.tile_pool(name="osb", bufs=2))
        p_ot = ctx.enter_context(tc.tile_pool(name="ot", bufs=2))
        p_ysb = ctx.enter_context(tc.tile_pool(name="ysb", bufs=2))

        cst = {}
        cst["xT"] = const.tile([128, 8, T], BF, name="xT_sb")
        nc.sync.dma_start(cst["xT"][:],
                          xT.ap().rearrange("(k p) t -> p k t", p=128))
        cst["xq"] = const.tile([128, 8, 512], BF, name="xq_sb")
        nc.sync.dma_start(cst["xq"][:],
                          xq.ap().rearrange("(k p) t -> p k t", p=128))
        cst["wqk"] = const.tile([128, 8, 2048], BF, name="wqk_sb")
        nc.sync.dma_start(cst["wqk"][:],
                          wqk.ap().rearrange("(k p) m -> p k m", p=128))
        cst["wv"] = const.tile([128, 8, 1040], BF, name="wv_sb")
        nc.sync.dma_start(cst["wv"][:],
                          wv.ap().rearrange("(k p) m -> p k m", p=128))
        cst["wr"] = const.tile([128, NP, H], BF, name="wr_sb")
        nc.sync.dma_start(cst["wr"][:], wr.ap().rearrange("h p m -> p h m"))
        cst["bqk"] = const.tile([128, 16], F32, name="bqk_sb")
        nc.sync.dma_start(cst["bqk"][:], bqkT.ap())
        cst["bv"] = const.tile([1, 1040], BF, name="bv_sb")
        nc.sync.dma_start(cst["bv"][:], bv.ap())
        cst["prot"] = const.tile([128, 128], BF, name="prot_sb")
        nc.sync.dma_start(cst["prot"][:], prot.ap())
        for nm, dt_ in (("cosK", cosK), ("sinK", sinK), ("cosQ", cosQ),
                        ("sinQ", sinQ), ("caus", caus)):
            cst[nm] = const.tile(list(dt_.shape), BF, name=f"{nm}_sb")
            nc.sync.dma_start(cst[nm][:], dt_.ap())
        cst["km"] = const.tile([128, NKC], F32, name="km_sb")
        nc.sync.dma_start(cst["km"][:], kmT.ap())
        cst["ones1"] = const.tile([1, 128], BF, name="ones1")
        nc.vector.memset(cst["ones1"][:], 1.0)
        cst["ones64"] = const.tile([128, 64], F32, name="ones64")
        nc.vector.memset(cst["ones64"][:], 1.0)

        v_sb = const.tile([128, NKC, NH * 65], BF, name="v_sb")
        qf = const.tile([128, NP, 512], BF, name="qf_sb")
        onp = const.tile([128, NP, 512], BF, name="onp_sb")

        pools = dict(
            consts=cst, ps_sp=ps_sp, ps_mm=ps_mm, ps_oa=ps_oa,
            kraw=p_kraw, t2=p_t2, kz=p_kz, qraw=p_qraw, E=p_E,
            osb=p_osb, ot=p_ot, ysb=p_ysb,
        )
        io = dict(y=yout.ap(), onp=onp, v_sb=v_sb, qf=qf)

        for _ in range(nreps):
            _emit_body(ctx, tc, io, pools, phases=phases)

    nc.compile()
    _PROGRAMS[key] = nc
    return nc


def make_inputs(hidden_state, attention_mask, w_qkv, b_qkv, w_out):
    """Host-side shard prep. Returns one input dict per core."""
    hidden_state = np.asarray(hidden_state)
    attention_mask = np.asarray(attention_mask)
    w_qkv = np.asarray(w_qkv)
    b_qkv = np.asarray(b_qkv)
    w_out = np.asarray(w_out)

    # rope tables (fp32 as in the reference, then bf16 for the device)
    inv_freq = 1.0 / (ROPE_BASE ** (np.arange(0, HD, 2, dtype=np.float32)
                                    / HD))
    t = np.arange(T, dtype=np.float32)
    freqs = np.outer(t, inv_freq)                      # [T, 32]
    emb = np.concatenate([freqs, freqs], axis=-1)      # [T, HD]
    cosT = np.cos(emb).T.astype(np.float32)            # [HD, T]
    sinT = np.sin(emb).T.astype(np.float32)
    sin_eff = sinT.copy()
    sin_eff[:32] = -sin_eff[:32]
    cos_pair = np.vstack([cosT, cosT]).astype(BF16)    # [128, T]
    sin_pair = np.vstack([sin_eff, sin_eff]).astype(BF16)

    # weight column orders
    cols_q = np.concatenate([np.arange(h * 192, h * 192 + 64)
                             for h in range(NH)])
    cols_k = cols_q + 64
    cols_v = cols_q + 128
    wqk = np.concatenate([w_qkv[:, cols_q], w_qkv[:, cols_k]],
                         axis=1).astype(BF16)          # [H, 2048]
    bqk = np.concatenate([b_qkv[cols_q], b_qkv[cols_k]]).astype(np.float32)
    bqkT = bqk.reshape(16, 128).T.copy()               # [128, 16]
    wv = np.zeros((H, 1040), dtype=BF16)
    bvr = np.zeros((1, 1040), dtype=BF16)
    for h in range(NH):
        wv[:, 65 * h:65 * h + 64] = w_qkv[:, cols_v[64 * h:64 * h + 64]]
        bvr[0, 65 * h:65 * h + 64] = b_qkv[cols_v[64 * h:64 * h + 64]]
        bvr[0, 65 * h + 64] = 1.0
    # rotate_half as a permutation matrix (sign folded into sin tables)
    prot = np.zeros((128, 128), dtype=BF16)
    for dst, srcr in ((0, 32), (32, 0), (64, 96), (96, 64)):
        for r in range(32):
            prot[srcr + r, dst + r] = 1.0
    wr = w_out.reshape(NP, 128, H).astype(BF16)

    dk = np.arange(128)[:, None]
    dq = np.arange(128)[None, :]

    in_maps = []
    for core in range(N_CORES):
        b, j = core // 4, core % 4
        qrows = np.concatenate([np.arange(128 * j + 512 * i,
                                          128 * j + 512 * i + 128)
                                for i in range(NQB)])
        # causal strips: within key chunk 4g+t of the diagonal group,
        # t < j all-pass, t == j diagonal, t > j fully masked
        caus = np.zeros((128, 4, 2, 128), dtype=BF16)
        for t in range(4):
            if t < j:
                pat = np.zeros((128, 128), dtype=BF16)
            elif t == j:
                pat = np.where(dq >= dk, 0.0, -1e4).astype(BF16)
            else:
                pat = np.full((128, 128), -1e4, dtype=BF16)
            caus[:, t, 0, :] = pat
            caus[:, t, 1, :] = pat

        kmT = (attention_mask[b].reshape(NKC, 128).T != 0) \
            .astype(np.float32)

        in_maps.append({
            "xT": np.ascontiguousarray(hidden_state[b].T).astype(BF16),
            "xq": np.ascontiguousarray(hidden_state[b][qrows].T)
            .astype(BF16),
            "wqk": wqk,
            "wv": wv,
            "wr": wr,
            "bqkT": bqkT,
            "bv": bvr,
            "prot": prot,
            "cosK": cos_pair,
            "sinK": sin_pair,
            "cosQ": np.ascontiguousarray(cos_pair[:, qrows]),
            "sinQ": np.ascontiguousarray(sin_pair[:, qrows]),
            "caus": caus.reshape(128, 1024),
            "kmT": kmT,
        })
    return in_maps


def kernel(hidden_state, attention_mask, w_qkv, b_qkv, w_out):
    nc = build_program(nreps=1)
    in_maps = make_inputs(hidden_state, attention_mask, w_qkv, b_qkv, w_out)
    res = run_bass_kernel_spmd(nc, in_maps, list(range(N_CORES))).results

    out = np.empty((B, T, H), dtype=np.float32)
    for core in range(N_CORES):
        b, j = core // 4, core % 4
        for i in range(NQB):
            out[b, 128 * j + 512 * i:128 * j + 512 * i + 128, :] = \
                res[core]["y"][128 * i:128 * i + 128, :]
    return out


# revision 4
# speedup vs baseline: 12.9510x; 10.8805x over previous
"""Multi-head causal attention (B=2, T=2048, H=1024, 16 heads) on 8 Trainium2
NeuronCores.

Sharding: data-parallel over batch (2 groups of 4 cores) x tensor-parallel over
heads (4 heads/core). Each core computes qkv projection for its heads, rotary
embedding, causal+padding-masked attention, and its partial out-projection;
a ReduceScatter over each 4-core group combines the out-proj partials, and the
host concatenates the shards.

Layout notes: scores are computed transposed (S^T: keys on partitions, queries
on the free axis) so softmax'd tiles feed the PV matmul directly as the
stationary operand without any transposes; the `[V | 1]` stationary trick makes
every PV matmul also produce the softmax row-sums. All matmuls are kept
full-tile (K=128 via zero-padded k-halves, N=512 outputs into exactly-sized
PSUM tiles) — partial/sliced matmul tiles hit a much slower path.

Self-contained: shapes/sharding hardcoded; only needs the concourse runtime.
"""
import sys

for _p in ("/opt/trn_rl_repo", "/root/.axon_site/_ro/trn_rl_repo"):
    if _p not in sys.path:
        sys.path.append(_p)

from contextlib import ExitStack

import numpy as np
import ml_dtypes

import concourse.bacc as bacc
import concourse.tile as tile
from concourse import mybir
from concourse.bass_utils import run_bass_kernel_spmd

BF16 = ml_dtypes.bfloat16
F32 = mybir.dt.float32
BF = mybir.dt.bfloat16

N_CORES = 8
B, T, H = 2, 2048, 1024
NH, HD = 16, 64
HPC = 4  # heads per core
NKC = T // 128  # 16 key chunks
NQT = T // 512  # 4 query tiles
ROPE_BASE = 10000.0
NEG = -1e30

_PROGRAMS = {}


def _emit_body(ctx, tc, io, pools, phases=(1, 2, 3, 4)):
    nc = tc.nc
    mult = mybir.AluOpType.mult
    add = mybir.AluOpType.add
    AF = mybir.ActivationFunctionType

    (xT_sb, wqk_sb, bqk_sb, wv_sb, bv_sb, wr_sb, cos_sb, sin_sb, caus_sb,
     km_sb, ones_bf, ones_f32, v_sb) = pools["consts"]
    ps_big = pools["ps_big"]
    ps_O = pools["ps_O"]
    ps_bc = pools["ps_bc"]
    p_qkraw = pools["qkraw"]
    p_rope = pools["rope"]
    p_qf = pools["qf"]
    p_on = pools["onorm"]
    p_E = pools["E"]
    p_Osb = pools["Osb"]
    p_ysb = pools["ysb"]
    y_int = io["y_int"]

    # ---- phase 1a: qk^T projection (pair-packed rows) + bias + rope -------
    # M-chunks: 0,1 = q pairs (heads 01, 23); 2,3 = k pairs.
    qf = []   # 2 pair-packed roped q tiles [128, T]
    kz = []   # 4 zero-padded roped k tiles [128, T] (one 64-row half live)
    for m in range(4 if 1 in phases else 0):
        qkraw = p_qkraw.tile([128, T], BF, tag="qkraw")
        for nt in range(NQT):
            ps = ps_big.tile([128, 512], F32, tag="big", name="psqk")
            for k in range(8):
                nc.tensor.matmul(
                    ps[:],
                    wqk_sb[:, k, 128 * m:128 * m + 128],
                    xT_sb[:, k, 512 * nt:512 * nt + 512],
                    start=(k == 0), stop=(k == 7),
                )
            nc.vector.tensor_scalar(
                qkraw[:, 512 * nt:512 * nt + 512], ps[:],
                bqk_sb[:, m:m + 1], None, add)
        # rotate_half as partition-block shifts (sign folded into sin table)
        sh = p_rope.tile([128, T], BF, tag="shift")
        for dst, src in ((0, 32), (32, 0), (64, 96), (96, 64)):
            nc.sync.dma_start(sh[dst:dst + 32, :], qkraw[src:src + 32, :])
        t1 = p_rope.tile([128, T], BF, tag="tmp")
        nc.vector.tensor_tensor(t1[:], qkraw[:], cos_sb[:], mult)
        t2 = p_rope.tile([128, T], BF, tag="tmp")
        nc.vector.tensor_tensor(t2[:], sh[:], sin_sb[:], mult)
        if m < 2:
            qfm = p_qf.tile([128, T], BF, tag="qf", name=f"qf{m}")
            nc.vector.tensor_tensor(qfm[:], t1[:], t2[:], add)
            qf.append(qfm)
        else:
            # k pair: split into two zero-padded per-head tiles so the
            # score matmuls run with a full K=128 stationary operand
            for e in range(2):
                kze = p_qf.tile([128, T], BF, tag="qf",
                                name=f"kz{m - 2}_{e}")
                lo, hi = 64 * e, 64 * e + 64
                nc.vector.memset(kze[64 - 64 * e:128 - 64 * e, :], 0.0)
                nc.vector.tensor_tensor(kze[lo:hi, :], t1[lo:hi, :],
                                        t2[lo:hi, :], add)
                kz.append(kze)

    # ---- phase 1b: v projection (natural layout, +ones column, +bias) ----
    # 4 key chunks share one 4-bank psum tile -> a single full-tile
    # evacuation each; the padding mask multiplies V rows (and the ones
    # column) to zero so masked keys vanish from both PV and the rowsums.
    for G in range(NKC // 4 if 2 in phases else 0):
        psv = ps_big.tile([128, 4, 512], F32, tag="big", name="psv")
        for j in range(4):
            qs = 4 * G + j
            # K=1 matmul adds the per-feature bias row and the ones columns
            nc.tensor.matmul(psv[:, j, :], ones_bf[0:1, :], bv_sb[:],
                             start=True, stop=False)
            for k in range(8):
                nc.tensor.matmul(
                    psv[:, j, :],
                    xT_sb[:, k, 128 * qs:128 * qs + 128],
                    wv_sb[:, k, :],
                    start=False, stop=(k == 7),
                )
        nc.vector.tensor_copy(v_sb[:, 4 * G:4 * G + 4, :], psv[:])
    for qs in range(NKC if 2 in phases else 0):
        nc.vector.tensor_scalar(v_sb[:, qs, :], v_sb[:, qs, :],
                                km_sb[:, qs:qs + 1], None, mult)

    # ---- phase 2: attention --------------------------------------------
    # onp[p]: normalized context for head pair p, pair-dim layout [128, T]
    onp = [p_on.tile([128, T], BF, tag="onp", name=f"onp{p}")
           for p in range(2)]

    for p in range(2 if 3 in phases else 0):  # head pairs
        qT = qf[p]
        # unnormalized context + rowsums for the whole pair, all q tiles
        Osb = [p_Osb.tile([65, T], F32, tag="Osb", name=f"Osb{e}")
               for e in range(2)]
        for nt in range(NQT):
            O_acc = [ps_O.tile([65, 512], F32, tag="Oacc", name=f"Oacc{e}")
                     for e in range(2)]
            nch = 4 * nt + 4
            # two key chunks share one 4-bank psum tile and one exp op
            for g in range(nch // 2):
                Sp = ps_big.tile([128, 2, 2, 512], F32, tag="big",
                                 name="Sp")
                for cc in range(2):
                    for e in range(2):
                        nc.tensor.matmul(
                            Sp[:, cc, e, :],
                            kz[2 * p + e][:, 128 * (2 * g + cc):
                                          128 * (2 * g + cc) + 128],
                            qT[:, 512 * nt:512 * nt + 512],
                            start=True, stop=True,
                        )
                E = p_E.tile([128, 2, 2, 512], BF, tag="E")
                nc.scalar.activation(E[:], Sp[:], AF.Exp, scale=0.125)
                if 2 * g >= 4 * nt:  # diagonal chunks: zero causal region
                    gg = (2 * g - 4 * nt) // 2
                    nc.gpsimd.tensor_tensor(
                        E[:], E[:],
                        caus_sb[:, 2048 * gg:2048 * gg + 2048]
                        .rearrange("p (a b c) -> p a b c", a=2, b=2),
                        mult)
                for cc in range(2):
                    c = 2 * g + cc
                    for e in range(2):
                        lh = 2 * p + e
                        nc.tensor.matmul(
                            O_acc[e][:],
                            v_sb[:, c, 65 * lh:65 * lh + 65],
                            E[:, cc, e, :],
                            start=(c == 0), stop=(c == nch - 1),
                        )
            for e in range(2):
                nc.vector.tensor_copy(Osb[e][:, 512 * nt:512 * nt + 512],
                                      O_acc[e][:])

        # normalize: O / rowsum (rowsum = row 64, from the ones column)
        for e in range(2):
            nc.vector.reciprocal(Osb[e][64:65, :], Osb[e][64:65, :])
            for half in range(2):
                hs = slice(1024 * half, 1024 * half + 1024)
                bc = ps_bc.tile([64, 1024], F32, tag="bc")
                for sub in range(2):
                    # ones row lives at partition 64 to match the rhs base
                    nc.tensor.matmul(
                        bc[:, 512 * sub:512 * sub + 512],
                        ones_f32[64:65, 0:64],
                        Osb[e][64:65, 1024 * half + 512 * sub:
                               1024 * half + 512 * sub + 512],
                        start=True, stop=True)
                if e == 0:
                    nc.vector.tensor_tensor(onp[p][0:64, hs],
                                            Osb[e][0:64, hs], bc[:], mult)
                else:
                    ot = p_ysb.tile([64, 1024], BF, tag="otmp",
                                    name="otmp")
                    nc.vector.tensor_tensor(ot[:], Osb[e][0:64, hs],
                                            bc[:], mult)
                    # odd head lives at partitions 64-127 of the pair tile
                    nc.sync.dma_start(onp[p][64:128, hs], ot[:])

    # ---- phase 3: out-projection partials -------------------------------
    for qs in range(NKC if 4 in phases else 0):
        for ns in range(2):
            py = ps_big.tile([128, 512], F32, tag="big", name="py")
            for p in range(2):
                nc.tensor.matmul(
                    py[:],
                    onp[p][:, 128 * qs:128 * qs + 128],
                    wr_sb[:, p, 512 * ns:512 * ns + 512],
                    start=(p == 0), stop=(p == 1),
                )
            ysb = p_ysb.tile([128, 512], BF, tag="ysb")
            nc.vector.tensor_copy(ysb[:], py[:])
            nc.sync.dma_start(
                y_int[128 * qs:128 * qs + 128, 512 * ns:512 * ns + 512],
                ysb[:])


def build_program(nreps=1, use_collective=True, phases=(1, 2, 3, 4)):
    key = (nreps, use_collective, tuple(phases))
    if key in _PROGRAMS:
        return _PROGRAMS[key]

    nc = bacc.Bacc("TRN2", target_bir_lowering=False, debug=False,
                   num_devices=N_CORES)
    xT = nc.dram_tensor("xT", [H, T], BF, kind="ExternalInput")
    wqk = nc.dram_tensor("wqk", [H, 512], BF, kind="ExternalInput")
    bqkT = nc.dram_tensor("bqkT", [128, 4], F32, kind="ExternalInput")
    wv = nc.dram_tensor("wv", [H, 512], BF, kind="ExternalInput")
    bv = nc.dram_tensor("bv", [1, 512], BF, kind="ExternalInput")
    wr = nc.dram_tensor("wr", [2, 128, H], BF, kind="ExternalInput")
    cosT = nc.dram_tensor("cosT", [128, T], BF, kind="ExternalInput")
    sinT = nc.dram_tensor("sinT", [128, T], BF, kind="ExternalInput")
    caus = nc.dram_tensor("caus", [128, 4096], BF, kind="ExternalInput")
    kmT = nc.dram_tensor("kmT", [128, NKC], F32, kind="ExternalInput")
    out_shape = [T // 4, H] if use_collective else [T, H]
    yout = nc.dram_tensor("y", out_shape, BF, kind="ExternalOutput")

    with tile.TileContext(nc) as tc, ExitStack() as ctx:
        const = ctx.enter_context(tc.tile_pool(name="const", bufs=1))
        ps_big = ctx.enter_context(tc.tile_pool(name="ps_big", bufs=1,
                                                space="PSUM"))
        ps_O = ctx.enter_context(tc.tile_pool(name="ps_O", bufs=2,
                                              space="PSUM"))
        ps_bc = ctx.enter_context(tc.tile_pool(name="ps_bc", bufs=1,
                                               space="PSUM"))
        p_qkraw = ctx.enter_context(tc.tile_pool(name="qkraw", bufs=2))
        p_rope = ctx.enter_context(tc.tile_pool(name="rope", bufs=2))
        p_qf = ctx.enter_context(tc.tile_pool(name="qf", bufs=6))
        p_on = ctx.enter_context(tc.tile_pool(name="onorm", bufs=2))
        p_E = ctx.enter_context(tc.tile_pool(name="E", bufs=2))
        p_Osb = ctx.enter_context(tc.tile_pool(name="Osb", bufs=2))
        p_ysb = ctx.enter_context(tc.tile_pool(name="ysb", bufs=3))
        dram = ctx.enter_context(tc.tile_pool(name="dram", bufs=1,
                                              space="DRAM"))

        # constant loads
        xT_sb = const.tile([128, 8, T], BF)
        nc.sync.dma_start(xT_sb[:], xT.ap().rearrange("(k p) t -> p k t",
                                                      p=128))
        wqk_sb = const.tile([128, 8, 512], BF)
        nc.sync.dma_start(wqk_sb[:], wqk.ap().rearrange("(k p) m -> p k m",
                                                        p=128))
        bqk_sb = const.tile([128, 4], F32)
        nc.sync.dma_start(bqk_sb[:], bqkT.ap())
        wv_sb = const.tile([128, 8, 512], BF)
        nc.sync.dma_start(wv_sb[:], wv.ap().rearrange("(k p) m -> p k m",
                                                      p=128))
        bv_sb = const.tile([1, 512], BF)
        nc.sync.dma_start(bv_sb[:], bv.ap())
        wr_sb = const.tile([128, 2, H], BF)
        nc.sync.dma_start(wr_sb[:], wr.ap().rearrange("h p m -> p h m"))
        cos_sb = const.tile([128, T], BF)
        nc.sync.dma_start(cos_sb[:], cosT.ap())
        sin_sb = const.tile([128, T], BF)
        nc.sync.dma_start(sin_sb[:], sinT.ap())
        caus_sb = const.tile([128, 4096], BF)
        nc.sync.dma_start(caus_sb[:], caus.ap())
        km_sb = const.tile([128, NKC], F32)
        nc.sync.dma_start(km_sb[:], kmT.ap())
        ones_bf = const.tile([1, 128], BF)
        nc.vector.memset(ones_bf[:], 1.0)
        ones_f32 = const.tile([128, 64], F32)
        nc.vector.memset(ones_f32[:], 1.0)
        v_sb = const.tile([128, NKC, 512], BF)

        y_int = dram.tile([T, H], BF, tag="yint")

        pools = dict(
            consts=(xT_sb, wqk_sb, bqk_sb, wv_sb, bv_sb, wr_sb, cos_sb,
                    sin_sb, caus_sb, km_sb, ones_bf, ones_f32, v_sb),
            ps_big=ps_big, ps_O=ps_O, ps_bc=ps_bc,
            qkraw=p_qkraw, rope=p_rope, qf=p_qf, onorm=p_on, E=p_E,
            Osb=p_Osb, ysb=p_ysb,
        )
        io = dict(y_int=y_int)

        for _ in range(nreps):
            _emit_body(ctx, tc, io, pools, phases=phases)

            if use_collective:
                rs_out = dram.tile([T // 4, H], BF, tag="rs")
                nc.gpsimd.collective_compute(
                    "ReduceScatter", mybir.AluOpType.add,
                    replica_groups=[[0, 1, 2, 3], [4, 5, 6, 7]],
                    ins=[y_int.opt()], outs=[rs_out.opt()],
                )
                nc.sync.dma_start(yout.ap(), rs_out[:])
            else:
                nc.sync.dma_start(yout.ap(), y_int[:])

    nc.compile()
    _PROGRAMS[key] = nc
    return nc


def make_inputs(hidden_state, attention_mask, w_qkv, b_qkv, w_out):
    """Host-side shard prep. Returns one input dict per core."""
    hidden_state = np.asarray(hidden_state)
    attention_mask = np.asarray(attention_mask)
    w_qkv = np.asarray(w_qkv)
    b_qkv = np.asarray(b_qkv)
    w_out = np.asarray(w_out)

    # rope tables (fp32 as in the reference, then bf16 for the device)
    inv_freq = 1.0 / (ROPE_BASE ** (np.arange(0, HD, 2, dtype=np.float32)
                                    / HD))
    t = np.arange(T, dtype=np.float32)
    freqs = np.outer(t, inv_freq)                      # [T, 32]
    emb = np.concatenate([freqs, freqs], axis=-1)      # [T, HD]
    cosT = np.cos(emb).T.astype(np.float32)            # [HD, T]
    sinT = np.sin(emb).T.astype(np.float32)
    sin_eff = sinT.copy()
    sin_eff[:32] = -sin_eff[:32]
    cos_pair = np.vstack([cosT, cosT]).astype(BF16)    # [128, T]
    sin_pair = np.vstack([sin_eff, sin_eff]).astype(BF16)

    # causal 0/1 strips, laid out per 2-chunk exp group:
    # group gg block = [pat(256gg) | pat(256gg) | pat(256gg+128) | ...]
    dk = np.arange(128)[:, None]
    dq = np.arange(512)[None, :]
    caus = np.zeros((128, 4096), dtype=BF16)
    for gg in range(2):
        for cc in range(2):
            pat = (dq >= dk + 256 * gg + 128 * cc).astype(BF16)
            base = 2048 * gg + 1024 * cc
            caus[:, base:base + 512] = pat
            caus[:, base + 512:base + 1024] = pat

    in_maps = []
    for core in range(N_CORES):
        b = core // 4
        hg = core % 4
        heads = [4 * hg + j for j in range(HPC)]

        cols_q = np.concatenate([np.arange(h * 192, h * 192 + 64)
                                 for h in heads])
        cols_k = cols_q + 64
        cols_v = cols_q + 128
        wqk = w_qkv[:, np.concatenate([cols_q, cols_k])].astype(BF16)
        bqk = b_qkv[np.concatenate([cols_q, cols_k])].astype(np.float32)
        bqkT = bqk.reshape(4, 128).T.copy()

        wv = np.zeros((H, 512), dtype=BF16)
        bv = np.zeros((1, 512), dtype=BF16)
        for j, h in enumerate(heads):
            wv[:, 65 * j:65 * j + 64] = w_qkv[:, cols_v[64 * j:64 * j + 64]]
            bv[0, 65 * j:65 * j + 64] = b_qkv[cols_v[64 * j:64 * j + 64]]
            bv[0, 65 * j + 64] = 1.0

        # pair-packed out-proj rows: wr[p] = rows of heads (2p, 2p+1)
        wr = w_out[256 * hg:256 * hg + 256, :].reshape(2, 128, H) \
            .astype(BF16)

        # 0/1 key-validity multiplier, folded into V and the ones column
        kmT = (attention_mask[b].reshape(NKC, 128).T != 0) \
            .astype(np.float32)

        in_maps.append({
            "xT": np.ascontiguousarray(hidden_state[b].T).astype(BF16),
            "wqk": np.ascontiguousarray(wqk),
            "bqkT": bqkT,
            "wv": wv,
            "bv": bv,
            "wr": wr,
            "cosT": cos_pair,
            "sinT": sin_pair,
            "caus": caus,
            "kmT": kmT,
        })
    return in_maps


def kernel(hidden_state, attention_mask, w_qkv, b_qkv, w_out,
           _use_collective=True):
    nc = build_program(nreps=1, use_collective=_use_collective)
    in_maps = make_inputs(hidden_state, attention_mask, w_qkv, b_qkv, w_out)
    res = run_bass_kernel_spmd(nc, in_maps, list(range(N_CORES))).results

    out = np.empty((B, T, H), dtype=np.float32)
    if _use_collective:
        for core in range(N_CORES):
            b, j = core // 4, core % 4
            out[b, 512 * j:512 * (j + 1), :] = \
                res[core]["y"].astype(np.float32)
    else:
        for b in range(B):
            out[b] = sum(res[4 * b + j]["y"].astype(np.float32)
                         for j in range(4))
    return out

